# revision 7
# baseline (speedup 1.0000x reference)
"""MoE (MiMoV2 FlashMoE) Trainium2 kernel: expert-parallel over 8 NeuronCores.

Strategy:
  Phase 1 (device): router — logits = x @ w_router.T in fp32, top-4 via
    exact max/mask iterations, combine weights = sigmoid normalized over
    the selected 4. Each core handles T/8 = 512 tokens.
  Host: compaction — per-expert token lists from combine > 0 (data
    movement only). Experts are sorted by load and assigned to
    (core, slot) so that slot j on every core has capacity cap[j] =
    max load among its 8 experts (rounded up). This cuts padded compute
    from 4*max_load to sum_j cap[j] (~17% less).
  Phase 2 (device): experts — 4 expert slots per core, all matmul
    operands bf16 (halves DMA vs f32; PE rate identical; rel err ~5e-3
    well under the 2e-2 gate). Per expert: G = Wg@X, U = Wu@X (contract
    over H in 8 chunks), H = silu(G)*U*combine (bf16), Y^T = Wd@H
    (contract over I in 6 chunks). Weights stationary; tokens moving.
    Each stationary 128x128 chunk is reused across the 2 column tiles.
  Host: scatter-add per-expert outputs into y [T, H].
"""
import math
import numpy as np
from contextlib import ExitStack

import concourse.bass as bass
import concourse.mybir as mybir
import concourse.tile as tile
from concourse import bacc
from concourse.bass_utils import run_bass_kernel_spmd

F32 = mybir.dt.float32
BF16 = mybir.dt.bfloat16
NPBF16 = mybir.dt.np(BF16)

# Problem shapes (hardcoded per contract)
E = 32          # experts
TOPK = 4
H = 1024        # hidden
I = 768         # intermediate
B, S = 2, 2048
T = B * S       # 4096 tokens
NCORES = 8
EPC = E // NCORES    # expert slots per core = 4
TPC = T // NCORES    # router tokens per core = 512
KH = H // 128        # 8 contraction chunks over H
KI = I // 128        # 6 contraction chunks over I

_program_cache = {}


def build_router(reps=1):
    """Per-core: logits^T = w_router @ x^T via PE (weights stationary, 512
    tokens moving), DVE 32x32 block transposes to [tokens, E], then a
    batched top-4 + combine-weight computation on a single [128, 4, E]
    tile. Selection compares exact fp32 logits."""
    nc = bacc.Bacc()
    NT = TPC // 128  # 4 token tiles
    xTc = nc.dram_tensor("xTc", [H, TPC], F32, kind="ExternalInput")
    wrT = nc.dram_tensor("wrT", [H, E], F32, kind="ExternalInput")
    comb_out = nc.dram_tensor("comb", [NT, 128, E], F32, kind="ExternalOutput")
    with ExitStack() as ctx:
        tc = ctx.enter_context(tile.TileContext(nc))
        sb = ctx.enter_context(tc.tile_pool(name="sb", bufs=1))
        work = ctx.enter_context(tc.tile_pool(name="work", bufs=2))
        ps = ctx.enter_context(tc.tile_pool(name="ps", bufs=2, space="PSUM"))

        xr = sb.tile([128, KH, TPC], F32)
        wr = sb.tile([128, KH, E], F32)
        for k in range(KH):
            nc.sync.dma_start(out=xr[:, k, :], in_=xTc[k * 128:(k + 1) * 128, :])
            nc.sync.dma_start(out=wr[:, k, :], in_=wrT[k * 128:(k + 1) * 128, :])

        for _ in range(reps):
            lgT_p = ps.tile([E, TPC], F32)   # logits^T, 1 PSUM bank
            for k in range(KH):
                nc.tensor.matmul(lgT_p, wr[:, k, :], xr[:, k, :],
                                 start=(k == 0), stop=(k == KH - 1))
            lgT = work.tile([E, TPC], F32)
            nc.vector.tensor_copy(lgT, lgT_p)
            # transpose to [128, NT, E] with DVE 32x32 block transposes
            lt = work.tile([128, NT, E], F32)
            for t in range(NT):
                for jb in range(128 // 32):
                    nc.vector.transpose(
                        lt[jb * 32:(jb + 1) * 32, t, :],
                        lgT[:, t * 128 + jb * 32: t * 128 + (jb + 1) * 32])
            # batched top-4: find 4th max per token via iterative masking
            cur = work.tile([128, NT, E], F32)
            nc.vector.tensor_copy(cur, lt)
            m = work.tile([128, NT, 1], F32)
            ge = work.tile([128, NT, E], F32)
            for _k in range(TOPK - 1):
                nc.vector.reduce_max(m, cur, axis=mybir.AxisListType.X)
                nc.vector.tensor_tensor(ge, cur, m.broadcast_to((128, NT, E)),
                                        op=mybir.AluOpType.is_ge)
                nc.vector.scalar_tensor_tensor(cur, ge, -1e30, cur,
                                               op0=mybir.AluOpType.mult,
                                               op1=mybir.AluOpType.add)
            nc.vector.reduce_max(m, cur, axis=mybir.AxisListType.X)
            # sel = (logits >= 4th max), combine = sel*sigmoid normalized
            sel = work.tile([128, NT, E], F32)
            nc.vector.tensor_tensor(sel, lt, m.broadcast_to((128, NT, E)),
                                    op=mybir.AluOpType.is_ge)
            sig = work.tile([128, NT, E], F32)
            nc.scalar.activation(sig, lt, mybir.ActivationFunctionType.Sigmoid)
            wsel = work.tile([128, NT, E], F32)
            nc.vector.tensor_mul(wsel, sel, sig)
            ssum = work.tile([128, NT, 1], F32)
            nc.vector.reduce_sum(ssum, wsel, axis=mybir.AxisListType.X)
            nc.vector.tensor_scalar_add(ssum, ssum, 1e-20)
            rsum = work.tile([128, NT, 1], F32)
            nc.vector.reciprocal(rsum, ssum)
            ct = work.tile([128, NT, E], F32)
            nc.vector.tensor_tensor(ct, wsel, rsum.broadcast_to((128, NT, E)),
                                    op=mybir.AluOpType.mult)
            for t in range(NT):
                nc.sync.dma_start(out=comb_out[t], in_=ct[:, t, :])
    nc.finalize()
    return nc


def build_experts(caps, reps=1):
    """Expert MLP kernel, all-bf16 matmuls, per-slot capacities.

    Per-core inputs (pre-laid-out on host so every DMA is contiguous):
      xg{j}  [128, KH, Cj]          bf16  xg[p,k,c] = x[tok_c, k*128+p]
      wgu{j} [KI, 128, 2, KH, 128]  bf16  [m,p,h,k,i] = w_{gate,up}[e, m*128+i, k*128+p]
      wd{j}  [KH2, 128, 2, KI, 128] bf16  [h2,p,i,k,o] = w_down[e, (2h2+i)*128+o, k*128+p]
      cw{j}  [1, Cj]                f32   combine weights (0 on padding)
    Output:
      yg{j}  [KH2, 128, 2, Cj]      bf16  [h2,p,i,c] = y^T[(2h2+i)*128+p, c]
    """
    nc = bacc.Bacc()
    KH2 = KH // 2
    xgs, wgus, wds, cws, ygs = [], [], [], [], []
    for j in range(EPC):
        C = caps[j]
        xgs.append(nc.dram_tensor(f"xg{j}", [128, KH, C], BF16,
                                  kind="ExternalInput"))
        wgus.append(nc.dram_tensor(f"wgu{j}", [KI, 128, 2, KH, 128], BF16,
                                   kind="ExternalInput"))
        wds.append(nc.dram_tensor(f"wd{j}", [KH2, 128, 2, KI, 128], BF16,
                                  kind="ExternalInput"))
        cws.append(nc.dram_tensor(f"cw{j}", [1, C], F32,
                                  kind="ExternalInput"))
        ygs.append(nc.dram_tensor(f"yg{j}", [KH2, 128, 2, C], BF16,
                                  kind="ExternalOutput"))
    warm_out = nc.dram_tensor("warm", [128, 1], F32, kind="ExternalOutput")

    with ExitStack() as ctx:
        tc = ctx.enter_context(tile.TileContext(nc))
        cwp = ctx.enter_context(tc.tile_pool(name="cwp", bufs=1))
        xgp = ctx.enter_context(tc.tile_pool(name="xgp", bufs=2))
        wgup = ctx.enter_context(tc.tile_pool(name="wgup", bufs=3))
        wdp = ctx.enter_context(tc.tile_pool(name="wdp", bufs=3))
        hp = ctx.enter_context(tc.tile_pool(name="hp", bufs=2))
        msc = ctx.enter_context(tc.tile_pool(name="msc", bufs=4))
        outp = ctx.enter_context(tc.tile_pool(name="outp", bufs=3))
        # PSUM budget (8 banks x 2KB): g0,g1,u0,u1 single-buffered (4) +
        # y0,y1 double-buffered (4) = 8. The warm-up tile reuses y0's slot.
        ps_gu = ctx.enter_context(tc.tile_pool(name="ps_gu", bufs=1,
                                               space="PSUM"))
        ps_d = ctx.enter_context(tc.tile_pool(name="ps_d", bufs=2,
                                              space="PSUM"))

        cwb = []
        for j in range(EPC):
            cwt = cwp.tile([128, caps[j]], F32, tag=f"cw{j}")
            nc.gpsimd.dma_start(out=cwt,
                                in_=cws[j][0:1, :].partition_broadcast(128))
            cwb.append(cwt)

        # PE warm-up while the first DMAs land (HAM 1.2 -> 2.4 GHz).
        wtile = cwp.tile([128, 512], F32, tag="warm")
        nc.vector.memset(wtile, 0.0)
        wps = ps_d.tile([128, 512], F32, tag="y0")
        for wi in range(6):
            nc.tensor.matmul(wps, wtile[:, :128], wtile,
                             start=(wi == 0), stop=(wi == 5))
        wres = cwp.tile([128, 1], F32, tag="warmres")
        nc.vector.tensor_copy(wres, wps[:, 0:1])
        nc.gpsimd.dma_start(out=warm_out[:], in_=wres)

        for _ in range(reps):
            for j in range(EPC):
                C = caps[j]
                cn0 = (C // 2 + 7) // 8 * 8
                cts = [(0, cn0), (cn0, C - cn0)]
                xg_t = xgp.tile([128, KH, C], BF16)
                nc.sync.dma_start(out=xg_t, in_=xgs[j][:])
                wgu_t = wgup.tile([128, KI, 2, KH, 128], BF16)
                for m in range(KI):
                    eng = nc.sync if m % 2 == 0 else nc.scalar
                    eng.dma_start(out=wgu_t[:, m], in_=wgus[j][m])
                h_t = hp.tile([128, KI, C], BF16)
                for m in range(KI):
                    pg, pu = [], []
                    for ci, (c0, cn) in enumerate(cts):
                        pg.append(ps_gu.tile([128, cn], F32, tag=f"g{ci}", name=f"pg{ci}"))
                        pu.append(ps_gu.tile([128, cn], F32, tag=f"u{ci}", name=f"pu{ci}"))
                    for half, pp in ((0, pg), (1, pu)):
                        for k in range(KH):
                            w = wgu_t[:, m, half, k, :]
                            for ci, (c0, cn) in enumerate(cts):
                                nc.tensor.matmul(pp[ci], w,
                                                 xg_t[:, k, c0:c0 + cn],
                                                 start=(k == 0),
                                                 stop=(k == KH - 1))
                    for ci, (c0, cn) in enumerate(cts):
                        sg = msc.tile([128, cn], F32, tag=f"sg{ci}")
                        nc.scalar.activation(sg, pg[ci],
                                             mybir.ActivationFunctionType.Silu)
                        t1 = msc.tile([128, cn], F32, tag=f"t1{ci}")
                        nc.vector.tensor_mul(t1, sg, pu[ci])
                        nc.vector.tensor_mul(h_t[:, m, c0:c0 + cn], t1,
                                             cwb[j][:, c0:c0 + cn])
                for h2 in range(KH2):
                    wd_t = wdp.tile([128, 2, KI, 128], BF16)
                    eng = nc.sync if h2 % 2 == 0 else nc.scalar
                    eng.dma_start(out=wd_t, in_=wds[j][h2])
                    yo = outp.tile([128, 2, C], BF16)
                    for i in range(2):
                        py = [ps_d.tile([128, cn], F32, tag=f"y{ci}",
                                        name=f"py{ci}",
                                        padded_shape=[128, 512])
                              for ci, (c0, cn) in enumerate(cts)]
                        for k in range(KI):
                            w = wd_t[:, i, k, :]
                            for ci, (c0, cn) in enumerate(cts):
                                nc.tensor.matmul(py[ci], w,
                                                 h_t[:, k, c0:c0 + cn],
                                                 start=(k == 0),
                                                 stop=(k == KI - 1))
                        for ci, (c0, cn) in enumerate(cts):
                            nc.vector.tensor_copy(yo[:, i, c0:c0 + cn], py[ci])
                    nc.gpsimd.dma_start(out=ygs[j][h2], in_=yo)
    nc.finalize()
    return nc


def _get_router():
    if "router" not in _program_cache:
        _program_cache["router"] = build_router()
    return _program_cache["router"]


def _get_experts(caps):
    key = ("experts", caps)
    if key not in _program_cache:
        _program_cache[key] = build_experts(caps)
    return _program_cache[key]


def prep_router_inputs(x):
    return np.ascontiguousarray(x.T)


def route_on_host(combine):
    """Per-expert token lists; sort experts by load; slot capacities."""
    idx = [np.nonzero(combine[:, e])[0] for e in range(E)]
    loads = np.array([len(ii) for ii in idx])
    order = np.argsort(-loads, kind="stable")          # experts, desc load
    # assign[j][c] = expert handled by core c, slot j
    assign = [[int(order[8 * j + c]) for c in range(NCORES)]
              for j in range(EPC)]
    caps = tuple(max(256, (int(loads[order[8 * j]]) + 15) // 16 * 16)
                 for j in range(EPC))
    return idx, assign, caps


def prep_expert_inputs(x, combine, idx, assign, caps, w_gate, w_up, w_down):
    """Per-core in_maps with tile-exact bf16 layouts (contiguous DMA)."""
    KH2 = KH // 2
    xb = x.astype(NPBF16)
    gb = w_gate.astype(NPBF16)
    ub = w_up.astype(NPBF16)
    db = w_down.astype(NPBF16)
    in_maps = []
    for c in range(NCORES):
        im = {}
        for j in range(EPC):
            C = caps[j]
            e = assign[j][c]
            ii = idx[e]
            n = len(ii)
            xg = np.zeros((128, KH, C), NPBF16)
            cwm = np.zeros((1, C), np.float32)
            if n:
                xg[:, :, :n] = xb[ii].reshape(n, KH, 128).transpose(2, 1, 0)
                cwm[0, :n] = combine[ii, e]
            g = gb[e].reshape(KI, 128, KH, 128)       # (m, i, k, p)
            u = ub[e].reshape(KI, 128, KH, 128)
            wgu = np.empty((KI, 128, 2, KH, 128), NPBF16)
            wgu[:, :, 0] = g.transpose(0, 3, 2, 1)    # (m, p, k, i)
            wgu[:, :, 1] = u.transpose(0, 3, 2, 1)
            d = db[e].reshape(KH2, 2, 128, KI, 128)   # (h2, i, o, k, p)
            wdh = np.ascontiguousarray(d.transpose(0, 4, 1, 3, 2))
            im[f"xg{j}"] = xg
            im[f"wgu{j}"] = np.ascontiguousarray(wgu)
            im[f"wd{j}"] = wdh
            im[f"cw{j}"] = cwm
        in_maps.append(im)
    return in_maps


def kernel(hidden_states, w_router, w_gate, w_up, w_down):
    x = np.ascontiguousarray(np.asarray(hidden_states, np.float32)).reshape(T, H)
    w_gate = np.asarray(w_gate, np.float32)
    w_up = np.asarray(w_up, np.float32)
    w_down = np.asarray(w_down, np.float32)
    xT = prep_router_inputs(x)
    wrT = np.ascontiguousarray(np.asarray(w_router, np.float32).T)   # [H, E]

    # ---- Phase 1: router on device ----
    nc1 = _get_router()
    in_maps1 = [
        {"xTc": np.ascontiguousarray(xT[:, c * TPC:(c + 1) * TPC]), "wrT": wrT}
        for c in range(NCORES)
    ]
    r1 = run_bass_kernel_spmd(nc1, in_maps1, list(range(NCORES)))
    combine = np.concatenate(
        [r1.results[c]["comb"].reshape(TPC, E) for c in range(NCORES)], axis=0)

    # ---- Host: compaction (data movement only) ----
    idx, assign, caps = route_on_host(combine)
    in_maps2 = prep_expert_inputs(x, combine, idx, assign, caps,
                                  w_gate, w_up, w_down)

    # ---- Phase 2: expert MLPs on device ----
    nc2 = _get_experts(caps)
    r2 = run_bass_kernel_spmd(nc2, in_maps2, list(range(NCORES)))

    # ---- Host: scatter-add ----
    KH2 = KH // 2
    y = np.zeros((T, H), np.float32)
    for c in range(NCORES):
        for j in range(EPC):
            C = caps[j]
            e = assign[j][c]
            ii = idx[e]
            n = len(ii)
            if n:
                # [KH2, 128(p), 2(i), C] -> [H, C]: H index = (2*h2+i)*128+p
                yt = r2.results[c][f"yg{j}"].astype(np.float32)
                yt = yt.transpose(0, 2, 1, 3).reshape(H, C)
                y[ii] += yt[:, :n].T
    return y.reshape(B, S, H)


# revision 10
# speedup vs baseline: 1.0412x; 1.0412x over previous
"""MoE (MiMoV2 FlashMoE) Trainium2 kernel: expert-parallel over 8 NeuronCores.

Strategy:
  Phase 1 (device): router — logits = x @ w_router.T in fp32, top-4 via
    exact max/mask iterations, combine weights = sigmoid normalized over
    the selected 4. Each core handles T/8 = 512 tokens.
  Host: compaction — per-expert token lists from combine > 0 (data
    movement only). Experts are sorted by load and assigned to
    (core, slot) so that slot j on every core has capacity cap[j] =
    max load among its 8 experts (rounded up). This cuts padded compute
    from 4*max_load to sum_j cap[j] (~17% less).
  Phase 2 (device): experts — 4 expert slots per core, all matmul
    operands bf16 (halves DMA vs f32; PE rate identical; rel err ~5e-3
    well under the 2e-2 gate). Per expert: G = Wg@X, U = Wu@X (contract
    over H in 8 chunks), H = silu(G)*U*combine (bf16), Y^T = Wd@H
    (contract over I in 6 chunks). Weights stationary; tokens moving.
    Each stationary 128x128 chunk is reused across the 2 column tiles.
  Host: scatter-add per-expert outputs into y [T, H].
"""
import math
import numpy as np
from contextlib import ExitStack

import concourse.bass as bass
import concourse.mybir as mybir
import concourse.tile as tile
from concourse import bacc
from concourse.bass_utils import run_bass_kernel_spmd

F32 = mybir.dt.float32
BF16 = mybir.dt.bfloat16
NPBF16 = mybir.dt.np(BF16)

# Problem shapes (hardcoded per contract)
E = 32          # experts
TOPK = 4
H = 1024        # hidden
I = 768         # intermediate
B, S = 2, 2048
T = B * S       # 4096 tokens
NCORES = 8
EPC = E // NCORES    # expert slots per core = 4
TPC = T // NCORES    # router tokens per core = 512
KH = H // 128        # 8 contraction chunks over H
KI = I // 128        # 6 contraction chunks over I

_program_cache = {}


def build_router(reps=1):
    """Per-core: logits^T = w_router @ x^T via PE (weights stationary, 512
    tokens moving), DVE 32x32 block transposes to [tokens, E], then a
    batched top-4 + combine-weight computation on a single [128, 4, E]
    tile. Selection compares exact fp32 logits."""
    nc = bacc.Bacc()
    NT = TPC // 128  # 4 token tiles
    xTc = nc.dram_tensor("xTc", [H, TPC], F32, kind="ExternalInput")
    wrT = nc.dram_tensor("wrT", [H, E], F32, kind="ExternalInput")
    comb_out = nc.dram_tensor("comb", [NT, 128, E], F32, kind="ExternalOutput")
    with ExitStack() as ctx:
        tc = ctx.enter_context(tile.TileContext(nc))
        sb = ctx.enter_context(tc.tile_pool(name="sb", bufs=1))
        work = ctx.enter_context(tc.tile_pool(name="work", bufs=2))
        ps = ctx.enter_context(tc.tile_pool(name="ps", bufs=2, space="PSUM"))

        xr = sb.tile([128, KH, TPC], F32)
        wr = sb.tile([128, KH, E], F32)
        for k in range(KH):
            nc.sync.dma_start(out=xr[:, k, :], in_=xTc[k * 128:(k + 1) * 128, :])
            nc.sync.dma_start(out=wr[:, k, :], in_=wrT[k * 128:(k + 1) * 128, :])

        for _ in range(reps):
            lgT_p = ps.tile([E, TPC], F32)   # logits^T, 1 PSUM bank
            for k in range(KH):
                nc.tensor.matmul(lgT_p, wr[:, k, :], xr[:, k, :],
                                 start=(k == 0), stop=(k == KH - 1))
            lgT = work.tile([E, TPC], F32)
            nc.vector.tensor_copy(lgT, lgT_p)
            # transpose to [128, NT, E] with DVE 32x32 block transposes
            lt = work.tile([128, NT, E], F32)
            for t in range(NT):
                for jb in range(128 // 32):
                    nc.vector.transpose(
                        lt[jb * 32:(jb + 1) * 32, t, :],
                        lgT[:, t * 128 + jb * 32: t * 128 + (jb + 1) * 32])
            # batched top-4: find 4th max per token via iterative masking
            cur = work.tile([128, NT, E], F32)
            nc.vector.tensor_copy(cur, lt)
            m = work.tile([128, NT, 1], F32)
            ge = work.tile([128, NT, E], F32)
            for _k in range(TOPK - 1):
                nc.vector.reduce_max(m, cur, axis=mybir.AxisListType.X)
                nc.vector.tensor_tensor(ge, cur, m.broadcast_to((128, NT, E)),
                                        op=mybir.AluOpType.is_ge)
                nc.vector.scalar_tensor_tensor(cur, ge, -1e30, cur,
                                               op0=mybir.AluOpType.mult,
                                               op1=mybir.AluOpType.add)
            nc.vector.reduce_max(m, cur, axis=mybir.AxisListType.X)
            # sel = (logits >= 4th max), combine = sel*sigmoid normalized
            sel = work.tile([128, NT, E], F32)
            nc.vector.tensor_tensor(sel, lt, m.broadcast_to((128, NT, E)),
                                    op=mybir.AluOpType.is_ge)
            sig = work.tile([128, NT, E], F32)
            nc.scalar.activation(sig, lt, mybir.ActivationFunctionType.Sigmoid)
            wsel = work.tile([128, NT, E], F32)
            nc.vector.tensor_mul(wsel, sel, sig)
            ssum = work.tile([128, NT, 1], F32)
            nc.vector.reduce_sum(ssum, wsel, axis=mybir.AxisListType.X)
            nc.vector.tensor_scalar_add(ssum, ssum, 1e-20)
            rsum = work.tile([128, NT, 1], F32)
            nc.vector.reciprocal(rsum, ssum)
            ct = work.tile([128, NT, E], F32)
            nc.vector.tensor_tensor(ct, wsel, rsum.broadcast_to((128, NT, E)),
                                    op=mybir.AluOpType.mult)
            for t in range(NT):
                nc.sync.dma_start(out=comb_out[t], in_=ct[:, t, :])
    nc.finalize()
    return nc


def build_experts(caps, reps=1):
    """Expert MLP kernel, all-bf16 matmuls, per-slot capacities.
    len(caps) = number of expert slots per core (4 for balanced routing).

    Per-core inputs (pre-laid-out on host so every DMA is contiguous):
      xg{j}  [128, KH, Cj]          bf16  xg[p,k,c] = x[tok_c, k*128+p]
      wgu{j} [KI, 128, 2, KH, 128]  bf16  [m,p,h,k,i] = w_{gate,up}[e, m*128+i, k*128+p]
      wd{j}  [KH2, 128, 2, KI, 128] bf16  [h2,p,i,k,o] = w_down[e, (2h2+i)*128+o, k*128+p]
      cw{j}  [1, Cj]                f32   combine weights (0 on padding)
    Output:
      yg{j}  [KH2, 128, 2, Cj]      bf16  [h2,p,i,c] = y^T[(2h2+i)*128+p, c]
    """
    nc = bacc.Bacc()
    KH2 = KH // 2
    nslots = len(caps)
    xgs, wgus, wds, cws, ygs = [], [], [], [], []
    for j in range(nslots):
        C = caps[j]
        xgs.append(nc.dram_tensor(f"xg{j}", [128, KH, C], BF16,
                                  kind="ExternalInput"))
        wgus.append(nc.dram_tensor(f"wgu{j}", [KI, 128, 2, KH, 128], BF16,
                                   kind="ExternalInput"))
        wds.append(nc.dram_tensor(f"wd{j}", [KH2, 128, 2, KI, 128], BF16,
                                  kind="ExternalInput"))
        cws.append(nc.dram_tensor(f"cw{j}", [1, C], F32,
                                  kind="ExternalInput"))
        ygs.append(nc.dram_tensor(f"yg{j}", [KH2, 128, 2, C], BF16,
                                  kind="ExternalOutput"))
    warm_out = nc.dram_tensor("warm", [128, 1], F32, kind="ExternalOutput")

    with ExitStack() as ctx:
        tc = ctx.enter_context(tile.TileContext(nc))
        cwp = ctx.enter_context(tc.tile_pool(name="cwp", bufs=1))
        xgp = ctx.enter_context(tc.tile_pool(name="xgp", bufs=2))
        wgup = ctx.enter_context(tc.tile_pool(name="wgup", bufs=3))
        wdp = ctx.enter_context(tc.tile_pool(name="wdp", bufs=3))
        hp = ctx.enter_context(tc.tile_pool(name="hp", bufs=2))
        msc = ctx.enter_context(tc.tile_pool(name="msc", bufs=2))
        outp = ctx.enter_context(tc.tile_pool(name="outp", bufs=3))
        # PSUM budget (8 banks x 2KB): g0,g1,u0,u1 single-buffered (4) +
        # y0,y1 double-buffered (4) = 8. The warm-up tile reuses y0's slot.
        ps_gu = ctx.enter_context(tc.tile_pool(name="ps_gu", bufs=1,
                                               space="PSUM"))
        ps_d = ctx.enter_context(tc.tile_pool(name="ps_d", bufs=2,
                                              space="PSUM"))

        cwb = []
        for j in range(nslots):
            cwt = cwp.tile([128, caps[j]], F32, tag=f"cw{j}")
            nc.gpsimd.dma_start(out=cwt,
                                in_=cws[j][0:1, :].partition_broadcast(128))
            cwb.append(cwt)

        # PE warm-up while the first DMAs land (HAM 1.2 -> 2.4 GHz).
        wtile = cwp.tile([128, 512], F32, tag="warm")
        nc.vector.memset(wtile, 0.0)
        wps = ps_d.tile([128, 512], F32, tag="y0")
        for wi in range(6):
            nc.tensor.matmul(wps, wtile[:, :128], wtile,
                             start=(wi == 0), stop=(wi == 5))
        wres = cwp.tile([128, 1], F32, tag="warmres")
        nc.vector.tensor_copy(wres, wps[:, 0:1])
        nc.gpsimd.dma_start(out=warm_out[:], in_=wres)

        for _ in range(reps):
            for j in range(nslots):
                C = caps[j]
                nct = (C + 511) // 512
                base = (C // nct + 7) // 8 * 8
                cts, off = [], 0
                while off < C:
                    cn = min(base, C - off)
                    cts.append((off, cn))
                    off += cn
                xg_t = xgp.tile([128, KH, C], BF16)
                nc.sync.dma_start(out=xg_t, in_=xgs[j][:])
                wgu_t = wgup.tile([128, KI, 2, KH, 128], BF16)
                for m in range(KI):
                    eng = nc.sync if m % 2 == 0 else nc.scalar
                    eng.dma_start(out=wgu_t[:, m], in_=wgus[j][m])
                h_t = hp.tile([128, KI, C], BF16)
                for m in range(KI):
                    pg, pu = [], []
                    for ci, (c0, cn) in enumerate(cts):
                        pg.append(ps_gu.tile([128, cn], F32, tag=f"g{ci % 2}", name=f"pg{ci}"))
                        pu.append(ps_gu.tile([128, cn], F32, tag=f"u{ci % 2}", name=f"pu{ci}"))
                    for half, pp in ((0, pg), (1, pu)):
                        for k in range(KH):
                            w = wgu_t[:, m, half, k, :]
                            for ci, (c0, cn) in enumerate(cts):
                                nc.tensor.matmul(pp[ci], w,
                                                 xg_t[:, k, c0:c0 + cn],
                                                 start=(k == 0),
                                                 stop=(k == KH - 1))
                    for ci, (c0, cn) in enumerate(cts):
                        sg = msc.tile([128, cn], F32, tag=f"sg{ci % 2}")
                        nc.scalar.activation(sg, pg[ci],
                                             mybir.ActivationFunctionType.Silu)
                        t1 = msc.tile([128, cn], F32, tag=f"t1{ci % 2}")
                        nc.vector.tensor_mul(t1, sg, pu[ci])
                        nc.vector.tensor_mul(h_t[:, m, c0:c0 + cn], t1,
                                             cwb[j][:, c0:c0 + cn])
                for h2 in range(KH2):
                    wd_t = wdp.tile([128, 2, KI, 128], BF16)
                    eng = nc.sync if h2 % 2 == 0 else nc.scalar
                    eng.dma_start(out=wd_t, in_=wds[j][h2])
                    yo = outp.tile([128, 2, C], BF16)
                    for i in range(2):
                        py = [ps_d.tile([128, cn], F32, tag=f"y{ci % 2}",
                                        name=f"py{ci}",
                                        padded_shape=[128, 512])
                              for ci, (c0, cn) in enumerate(cts)]
                        for k in range(KI):
                            w = wd_t[:, i, k, :]
                            for ci, (c0, cn) in enumerate(cts):
                                nc.tensor.matmul(py[ci], w,
                                                 h_t[:, k, c0:c0 + cn],
                                                 start=(k == 0),
                                                 stop=(k == KI - 1))
                        for ci, (c0, cn) in enumerate(cts):
                            nc.scalar.activation(
                                yo[:, i, c0:c0 + cn], py[ci],
                                mybir.ActivationFunctionType.Copy)
                    nc.gpsimd.dma_start(out=ygs[j][h2], in_=yo)
    nc.finalize()
    return nc


def _get_router():
    if "router" not in _program_cache:
        _program_cache["router"] = build_router()
    return _program_cache["router"]


def _get_experts(caps):
    key = ("experts", caps)
    if key not in _program_cache:
        _program_cache[key] = build_experts(caps)
    return _program_cache[key]


def prep_router_inputs(x):
    return np.ascontiguousarray(x.T)


MAXC = 1024    # per-slot token capacity limit (PSUM/SBUF sizing)


def route_on_host(combine):
    """Split experts into <=MAXC-token pieces, sort by size, and pack into
    slots of 8 (one piece per core) so slot j's capacity is the max piece
    length in that group. For the typical balanced routing this yields
    EPC=4 slots with caps ~ the sorted per-expert loads."""
    idx = [np.nonzero(combine[:, e])[0] for e in range(E)]
    pieces = []                       # (expert, lo, n) over idx[e][lo:lo+n]
    for e in range(E):
        n = len(idx[e])
        nparts = max(1, -(-n // MAXC))
        base = -(-n // nparts)
        for lo in range(0, n, base):
            pieces.append((e, lo, min(base, n - lo)))
    pieces.sort(key=lambda p: -p[2])
    while len(pieces) % NCORES:
        pieces.append((0, 0, 0))      # empty filler piece
    nslots = len(pieces) // NCORES
    assign = [[pieces[NCORES * j + c] for c in range(NCORES)]
              for j in range(nslots)]
    caps = tuple(max(256, (max(p[2] for p in assign[j]) + 7) // 8 * 8)
                 for j in range(nslots))
    return idx, assign, caps


def prep_expert_inputs(x, combine, idx, assign, caps, w_gate, w_up, w_down):
    """Per-core in_maps with tile-exact bf16 layouts (contiguous DMA)."""
    KH2 = KH // 2
    xb = x.astype(NPBF16)
    gb = w_gate.astype(NPBF16)
    ub = w_up.astype(NPBF16)
    db = w_down.astype(NPBF16)
    in_maps = []
    for c in range(NCORES):
        im = {}
        for j in range(len(caps)):
            C = caps[j]
            e, lo, n = assign[j][c]
            ii = idx[e][lo:lo + n]
            xg = np.zeros((128, KH, C), NPBF16)
            cwm = np.zeros((1, C), np.float32)
            if n:
                xg[:, :, :n] = xb[ii].reshape(n, KH, 128).transpose(2, 1, 0)
                cwm[0, :n] = combine[ii, e]
            g = gb[e].reshape(KI, 128, KH, 128)       # (m, i, k, p)
            u = ub[e].reshape(KI, 128, KH, 128)
            wgu = np.empty((KI, 128, 2, KH, 128), NPBF16)
            wgu[:, :, 0] = g.transpose(0, 3, 2, 1)    # (m, p, k, i)
            wgu[:, :, 1] = u.transpose(0, 3, 2, 1)
            d = db[e].reshape(KH2, 2, 128, KI, 128)   # (h2, i, o, k, p)
            wdh = np.ascontiguousarray(d.transpose(0, 4, 1, 3, 2))
            im[f"xg{j}"] = xg
            im[f"wgu{j}"] = np.ascontiguousarray(wgu)
            im[f"wd{j}"] = wdh
            im[f"cw{j}"] = cwm
        in_maps.append(im)
    return in_maps


def kernel(hidden_states, w_router, w_gate, w_up, w_down):
    x = np.ascontiguousarray(np.asarray(hidden_states, np.float32)).reshape(T, H)
    w_gate = np.asarray(w_gate, np.float32)
    w_up = np.asarray(w_up, np.float32)
    w_down = np.asarray(w_down, np.float32)
    xT = prep_router_inputs(x)
    wrT = np.ascontiguousarray(np.asarray(w_router, np.float32).T)   # [H, E]

    # ---- Phase 1: router on device ----
    nc1 = _get_router()
    in_maps1 = [
        {"xTc": np.ascontiguousarray(xT[:, c * TPC:(c + 1) * TPC]), "wrT": wrT}
        for c in range(NCORES)
    ]
    r1 = run_bass_kernel_spmd(nc1, in_maps1, list(range(NCORES)))
    combine = np.concatenate(
        [r1.results[c]["comb"].reshape(TPC, E) for c in range(NCORES)], axis=0)

    # ---- Host: compaction (data movement only) ----
    idx, assign, caps = route_on_host(combine)
    in_maps2 = prep_expert_inputs(x, combine, idx, assign, caps,
                                  w_gate, w_up, w_down)

    # ---- Phase 2: expert MLPs on device ----
    nc2 = _get_experts(caps)
    r2 = run_bass_kernel_spmd(nc2, in_maps2, list(range(NCORES)))

    # ---- Host: scatter-add ----
    KH2 = KH // 2
    y = np.zeros((T, H), np.float32)
    for c in range(NCORES):
        for j in range(len(caps)):
            C = caps[j]
            e, lo, n = assign[j][c]
            ii = idx[e][lo:lo + n]
            if n:
                # [KH2, 128(p), 2(i), C] -> [H, C]: H index = (2*h2+i)*128+p
                yt = r2.results[c][f"yg{j}"].astype(np.float32)
                yt = yt.transpose(0, 2, 1, 3).reshape(H, C)
                y[ii] += yt[:, :n].T
    return y.reshape(B, S, H)
